# revision 21
# baseline (speedup 1.0000x reference)
"""Trainium2 Bass kernel for nn_Net_58712202936903 (dense_mlp).

Math restructuring (validated vs reference to ~5e-7 rel):
  Network: y = (W1 x + b1) * (W2 x + b2)            (layer 1, "SKIP")
           h = (A y + b3) * (B y + b4)              (layer 2, "MUL")
           v = w_out . h
  Layer2+output collapse to a quadratic form in y:
           v(y) = y^T C y + d^T y + e
      C = A^T diag(w) B,  d = B^T (w*b3) + A^T (w*b4),  e = b3 . (w*b4)
  JVP (grad term): v_grad = u1^T (S y + d),  S = C + C^T,
      u1 = z1*z2d + z2*z1d computed EXACTLY via the central difference of
      the bilinear layer-1 product: 2*u1 = y(x+xd) - y(x-xd).

Distribution: pure data parallel over 8 cores; batch 65536 -> 8192/core,
center 512 -> 64/core. Weights replicated; identical SPMD program per core.
"""

import os
import numpy as np

P = 128
NCORES = 8
BATCH = 65536
PER_CORE = BATCH // NCORES      # 8192
NT = 512                        # columns per compute tile
NTILES = PER_CORE // NT         # 16
CPER = 512 // NCORES            # center cols per core = 64

# float32r single-pass PE mode (reduced precision) for main-loop matmuls.
F32R = os.environ.get("KB_F32R", "0") == "1"


def _build_program(ntiles=NTILES, nt=NT, ncenter=CPER):
    import concourse.bass as bass
    import concourse.tile as tile
    from concourse import bacc, mybir

    f32 = mybir.dt.float32
    AF = mybir.ActivationFunctionType
    OP = mybir.AluOpType

    nc = bacc.Bacc("TRN2", target_bir_lowering=False, debug=False)
    fmm = mybir.dt.float32r if F32R else f32
    bf16 = mybir.dt.bfloat16
    KL1 = 27                    # bf16x2 K-stack: [Wh;Wh;Wl] . [xh;xl;xh]

    NDF = 2
    nd = ntiles // NDF          # DMA macro-tiles
    nt2 = NDF * nt

    # ---- DRAM parameters (per-core) ----
    stkA = nc.dram_tensor("stkA", [nd, 4, KL1, nt2], bf16, kind="ExternalInput").ap()
    stkB = nc.dram_tensor("stkB", [nd, 4, KL1, nt2], bf16, kind="ExternalInput").ap()
    wstk = nc.dram_tensor("wstk", [4, KL1, P], bf16, kind="ExternalInput").ap()
    amat = nc.dram_tensor("amat", [P, P], f32, kind="ExternalInput").ap()
    bmat = nc.dram_tensor("bmat", [P, P], f32, kind="ExternalInput").ap()
    wvec = nc.dram_tensor("wvec", [P, 1], f32, kind="ExternalInput").ap()
    b3v = nc.dram_tensor("b3v", [P, 1], f32, kind="ExternalInput").ap()
    b4v = nc.dram_tensor("b4v", [P, 1], f32, kind="ExternalInput").ap()
    ident = nc.dram_tensor("ident", [P, P], f32, kind="ExternalInput").ap()
    onesc = nc.dram_tensor("onesc", [P, 2], fmm, kind="ExternalInput").ap()
    redw = nc.dram_tensor("redw", [P, 64], fmm, kind="ExternalInput").ap()
    maskr = nc.dram_tensor("maskr", [1, P], f32, kind="ExternalInput").ap()
    ctr = nc.dram_tensor("ctr", [KL1, ncenter], bf16, kind="ExternalInput").ap()

    vt_o = nc.dram_tensor("vt_o", [1, ntiles * nt], f32, kind="ExternalOutput").ap()
    vy_o = nc.dram_tensor("vy_o", [1, ntiles * nt], f32, kind="ExternalOutput").ap()
    vg_o = nc.dram_tensor("vg_o", [1, ntiles * nt], f32, kind="ExternalOutput").ap()
    vc_o = nc.dram_tensor("vc_o", [1, ncenter], f32, kind="ExternalOutput").ap()

    with tile.TileContext(nc) as tc:
        with tc.tile_pool(name="const", bufs=1) as cpool:
          with (
            tc.tile_pool(name="prepsum", bufs=2, space="PSUM") as prepsum,
            tc.tile_pool(name="presb", bufs=1) as presb,
          ):
            # ---- constants into SBUF ----
            wst = cpool.tile([P, P], bf16)
            for g in range(4):
                nc.sync.dma_start(wst[32 * g:32 * g + KL1, :], wstk[g])
            w2low = cpool.tile([KL1, P], bf16)
            nc.sync.dma_start(w2low[:], wstk[1])
            A_sb = cpool.tile([P, P], f32)
            nc.sync.dma_start(A_sb[:], amat)
            B_sb = cpool.tile([P, P], f32)
            nc.sync.dma_start(B_sb[:], bmat)
            w_sb = cpool.tile([P, 1], f32)
            nc.sync.dma_start(w_sb[:], wvec)
            b3_sb = cpool.tile([P, 1], f32)
            nc.sync.dma_start(b3_sb[:], b3v)
            b4_sb = cpool.tile([P, 1], f32)
            nc.sync.dma_start(b4_sb[:], b4v)
            I_sb = cpool.tile([P, P], f32)
            nc.sync.dma_start(I_sb[:], ident)
            onespair_sb = cpool.tile([P, 2], fmm)
            nc.sync.dma_start(onespair_sb[:], onesc)
            ones_sb = onespair_sb[:, 0:1]
            halfones_sb = onespair_sb[:, 1:2]
            redw_sb = cpool.tile([P, 64], fmm)
            nc.sync.dma_start(redw_sb[:], redw)
            onespad = redw_sb[:, 0:32]
            halfpad = redw_sb[:, 32:64]
            maskrow_sb = cpool.tile([1, P], f32)
            nc.sync.dma_start(maskrow_sb[:], maskr)

            # ---- one-time weight transforms on device ----
            _pc = [0]

            def pre_ps(shape):
                _pc[0] += 1
                t = prepsum.tile([P, P], f32, tag="pps", name=f"pps{_pc[0]}")
                return t[: shape[0], : shape[1]]

            # wB = diag(w) @ B
            wB_sb = presb.tile([P, P], f32)
            nc.vector.scalar_tensor_tensor(
                wB_sb[:], B_sb[:], 1.0, w_sb[:].to_broadcast((P, P)), OP.mult, OP.mult
            )
            # C = A^T (diag(w) B)
            C_ps = pre_ps((P, P))
            nc.tensor.matmul(C_ps, A_sb[:], wB_sb[:], start=True, stop=True)
            C_sb = presb.tile([P, P], f32)
            nc.vector.tensor_copy(C_sb[:], C_ps)
            CT_ps = pre_ps((P, P))
            nc.tensor.transpose(CT_ps, C_sb[:], I_sb[:])
            CT_sb = cpool.tile([P, P], fmm)     # lhsT for r = C y
            nc.vector.tensor_copy(CT_sb[:], CT_ps)
            CTf_sb = presb.tile([P, P], f32)
            nc.vector.tensor_copy(CTf_sb[:], CT_ps)
            S_sb = cpool.tile([P, P], f32)      # = C + C^T (symmetric)
            nc.vector.tensor_tensor(S_sb[:], C_sb[:], CTf_sb[:], OP.add)
            # bf16x2 splits of the L2 stationaries (lhsT = C^T and S)
            Ch_sb = cpool.tile([P, P], bf16)
            nc.vector.tensor_copy(Ch_sb[:], CTf_sb[:])
            Cl_sb = cpool.tile([P, P], bf16)
            nc.vector.tensor_tensor(Cl_sb[:], CTf_sb[:], Ch_sb[:], OP.subtract)
            Sh_sb = cpool.tile([P, P], bf16)
            nc.vector.tensor_copy(Sh_sb[:], S_sb[:])
            Sl_sb = cpool.tile([P, P], bf16)
            nc.vector.tensor_tensor(Sl_sb[:], S_sb[:], Sh_sb[:], OP.subtract)
            # d = B^T (w*b3) + A^T (w*b4)
            wb3_sb = presb.tile([P, 1], f32)
            nc.vector.tensor_tensor(wb3_sb[:], b3_sb[:], w_sb[:], OP.mult)
            wb4_sb = presb.tile([P, 1], f32)
            nc.vector.tensor_tensor(wb4_sb[:], b4_sb[:], w_sb[:], OP.mult)
            d_ps = pre_ps((P, 1))
            nc.tensor.matmul(d_ps, B_sb[:], wb3_sb[:], start=True, stop=False)
            nc.tensor.matmul(d_ps, A_sb[:], wb4_sb[:], start=False, stop=True)
            d_sb = cpool.tile([P, 1], f32)
            nc.vector.tensor_copy(d_sb[:], d_ps)
            # e = b3 . (w*b4); ebias has e on partitions 0..63, 0 above
            # (so one ACT applies +e to v_t/v_y rows 0/32 and +0 to v_grad row 64)
            e_ps = pre_ps((1, 1))
            nc.tensor.matmul(e_ps, b3_sb[:], wb4_sb[:], start=True, stop=True)
            e_sb = presb.tile([1, 1], f32)
            nc.vector.tensor_copy(e_sb[:], e_ps)
            ebias_ps = pre_ps((P, 1))
            nc.tensor.matmul(ebias_ps, maskrow_sb[:], e_sb[:], start=True, stop=True)
            ebias_sb = cpool.tile([P, 1], f32)
            nc.vector.tensor_copy(ebias_sb[:], ebias_ps)

          with (
            tc.tile_pool(name="xin", bufs=4) as xpool,
            tc.tile_pool(name="zp", bufs=4, space="PSUM") as zpool,
            tc.tile_pool(name="rp", bufs=3, space="PSUM") as rpool,
            tc.tile_pool(name="vp", bufs=1, space="PSUM") as vpool,
            tc.tile_pool(name="pr", bufs=9) as ppool,
            tc.tile_pool(name="sb", bufs=3) as spool,
            tc.tile_pool(name="vout", bufs=1) as opool,
          ):
            vout = opool.tile([65, ntiles * nt], f32)
            vcout = opool.tile([1, ncenter], f32)

            HB = min(4, nd)             # DMA macro-tiles per phase block
            for blk in range(nd // HB):
                prods = {}
                # ---------- phase A: layer 1 + Hadamard products ----------
                for dj in range(HB):
                    di = blk * HB + dj
                    xa = xpool.tile([P, nt2], bf16, tag="xa")
                    xb = xpool.tile([P, nt2], bf16, tag="xb")
                    for g in range(4):
                        nc.sync.dma_start(xa[32 * g:32 * g + KL1, :], stkA[di, g])
                        nc.sync.dma_start(xb[32 * g:32 * g + KL1, :], stkB[di, g])
                    for j in range(NDF):
                        i = NDF * di + j
                        cs = slice(j * nt, (j + 1) * nt)
                        zA = []
                        for g in range(4):
                            z = zpool.tile([P, nt], f32, tag="z", name=f"zA{g}_{i}")
                            nc.tensor.matmul(
                                z[:], wst[32 * g:32 * g + KL1, :],
                                xa[32 * g:32 * g + KL1, cs],
                                start=True, stop=True, tile_position=(32 * g, 0),
                            )
                            zA.append(z)
                        c1 = spool.tile([P, nt], f32, tag="c1")
                        nc.scalar.activation(c1[:], zA[1][:], AF.Copy)
                        yl = ppool.tile([P, nt], fmm, tag="yl", name=f"yl_{i}")
                        nc.vector.tensor_tensor(yl[:], zA[0][:], c1[:], OP.mult)
                        c3 = spool.tile([P, nt], f32, tag="c3")
                        nc.scalar.activation(c3[:], zA[3][:], AF.Copy)
                        Pp = spool.tile([P, nt], f32, tag="Pp")
                        nc.vector.tensor_tensor(Pp[:], zA[2][:], c3[:], OP.mult)

                        zB = []
                        for g in range(4):
                            z = zpool.tile([P, nt], f32, tag="z", name=f"zB{g}_{i}")
                            nc.tensor.matmul(
                                z[:], wst[32 * g:32 * g + KL1, :],
                                xb[32 * g:32 * g + KL1, cs],
                                start=True, stop=True, tile_position=(32 * g, 0),
                            )
                            zB.append(z)
                        c5 = spool.tile([P, nt], f32, tag="c5")
                        nc.scalar.activation(c5[:], zB[1][:], AF.Copy)
                        c6 = spool.tile([P, nt], f32, tag="c6")
                        nc.scalar.activation(c6[:], zB[0][:], AF.Copy)
                        Mm = spool.tile([P, nt], f32, tag="Mm")
                        nc.gpsimd.tensor_tensor(Mm[:], c6[:], c5[:], OP.mult)
                        c7 = spool.tile([P, nt], f32, tag="c7")
                        nc.scalar.activation(c7[:], zB[3][:], AF.Copy)
                        yT = ppool.tile([P, nt], fmm, tag="yT", name=f"yT_{i}")
                        nc.vector.tensor_tensor(yT[:], zB[2][:], c7[:], OP.mult)

                        # u2 = Pp - Mm = 2*u1 (SBUF only -> GpSimd)
                        u2 = ppool.tile([P, nt], f32, tag="u2", name=f"u2_{i}")
                        nc.gpsimd.tensor_tensor(u2[:], Pp[:], Mm[:], OP.subtract)
                        prods[i] = (yl, yT, u2)

                # ---------- phase B: layer-2 matvecs, w, reductions ----------
                for dj in range(HB):
                    for j in range(NDF):
                        i = NDF * (blk * HB + dj) + j
                        sl = slice(i * nt, (i + 1) * nt)
                        yl, yT, u2 = prods[i]
                        rT = rpool.tile([P, nt], f32, tag="r", name=f"rT_{i}")
                        nc.tensor.matmul(rT[:], CT_sb[:], yT[:],
                                         start=True, stop=True)
                        wT = spool.tile([P, nt], fmm, tag="wT")
                        nc.vector.scalar_tensor_tensor(
                            wT[:], rT[:], d_sb[:], yT[:], OP.add, OP.mult
                        )
                        rY = rpool.tile([P, nt], f32, tag="r", name=f"rY_{i}")
                        nc.tensor.matmul(rY[:], CT_sb[:], yl[:],
                                         start=True, stop=True)
                        wY = spool.tile([P, nt], fmm, tag="wY")
                        nc.vector.scalar_tensor_tensor(
                            wY[:], rY[:], d_sb[:], yl[:], OP.add, OP.mult
                        )
                        rG = rpool.tile([P, nt], f32, tag="r", name=f"rG_{i}")
                        nc.tensor.matmul(rG[:], S_sb[:], yl[:],
                                         start=True, stop=True)
                        wG = spool.tile([P, nt], fmm, tag="wG")
                        nc.vector.scalar_tensor_tensor(
                            wG[:], rG[:], d_sb[:], u2[:], OP.add, OP.mult
                        )

                        # reductions, col-tiled into one PSUM bank
                        psV = vpool.tile([P, nt], f32, tag="v", name=f"v_{i}")
                        nc.tensor.matmul(psV[0:32, :], onespad, wT[:],
                                         start=True, stop=True,
                                         tile_position=(0, 0))
                        nc.tensor.matmul(psV[32:64, :], onespad, wY[:],
                                         start=True, stop=True,
                                         tile_position=(0, 32))
                        nc.tensor.matmul(psV[64:96, :], halfpad, wG[:],
                                         start=True, stop=True,
                                         tile_position=(0, 64))
                        nc.scalar.activation(
                            vout[0:65, sl], psV[0:65, :],
                            AF.Identity, bias=ebias_sb[0:65, :],
                        )

            # ---- center stream (tiny) ----
            xc = xpool.tile([KL1, ncenter], bf16, tag="xc")
            nc.sync.dma_start(xc[:], ctr)
            zc1 = zpool.tile([P, nt], f32, tag="z")
            zc2 = zpool.tile([P, nt], f32, tag="z")
            nc.tensor.matmul(zc1[:, :ncenter], wst[0:KL1, :], xc[:],
                             start=True, stop=True, tile_position=(0, 0))
            nc.tensor.matmul(zc2[:, :ncenter], w2low[:], xc[:],
                             start=True, stop=True, tile_position=(0, 0))
            cc = spool.tile([P, nt], f32, tag="c1")
            nc.scalar.activation(cc[:, :ncenter], zc2[:, :ncenter], AF.Copy)
            yc = spool.tile([P, nt], fmm, tag="yl")
            nc.vector.tensor_tensor(
                yc[:, :ncenter], zc1[:, :ncenter], cc[:, :ncenter], OP.mult
            )
            rc = rpool.tile([P, nt], f32, tag="r")
            nc.tensor.matmul(rc[:, :ncenter], CT_sb[:], yc[:, :ncenter],
                             start=True, stop=True)
            wc = spool.tile([P, nt], fmm, tag="wT")
            nc.vector.scalar_tensor_tensor(
                wc[:, :ncenter], rc[:, :ncenter], d_sb[:], yc[:, :ncenter],
                OP.add, OP.mult,
            )
            psC = rpool.tile([P, nt], f32, tag="r")
            nc.tensor.matmul(psC[0:1, :ncenter], ones_sb, wc[:, :ncenter],
                             start=True, stop=True)
            nc.scalar.activation(
                vcout[0:1, :], psC[0:1, :ncenter],
                AF.Identity, bias=ebias_sb[0:1, :],
            )

            # ---- output DMAs ----
            nc.sync.dma_start(vt_o, vout[0:1, :])
            nc.sync.dma_start(vy_o, vout[32:33, :])
            nc.sync.dma_start(vg_o, vout[64:65, :])
            nc.sync.dma_start(vc_o, vcout[:])

    nc.compile()
    return nc


def _prep_inputs(T, l, l1_dot, center, w1_1, b1_1, w2_1, b2_1,
                 w1_2, b1_2, w2_2, b2_2, w_out,
                 ntiles=NTILES, nt=NT, ncenter=CPER):
    """Host-side shard + layout prep. Returns list of per-core input dicts."""
    import ml_dtypes
    f32 = np.float32
    bf = ml_dtypes.bfloat16
    NDF = 2
    nd, nt2 = ntiles // NDF, NDF * nt

    def bsplit(x):  # -> (hi, lo) bf16 arrays
        hi = x.astype(bf)
        lo = (x - hi.astype(f32)).astype(bf)
        return hi, lo

    def aug_t(x):  # (n, 8) -> (9, n) with ones row
        n = x.shape[0]
        out = np.empty((9, n), f32)
        out[0:8] = x.T
        out[8] = 1.0
        return out

    W1a = np.concatenate([w1_1, b1_1[:, None]], axis=1).astype(f32)  # (128, 9)
    W2a = np.concatenate([w2_1, b2_1[:, None]], axis=1).astype(f32)

    def wstack(W):  # (128, 9) -> (27, 128) bf16 [Wh; Wh; Wl]
        hi, lo = bsplit(W.T)
        return np.concatenate([hi, hi, lo], axis=0)

    w1s, w2s = wstack(W1a), wstack(W2a)
    wstk = np.stack([w1s, w2s, w1s, w2s])  # (4, 27, 128) bf16
    wvec = w_out.reshape(P, 1).astype(f32)
    b3v = b1_2.reshape(P, 1).astype(f32)
    b4v = b2_2.reshape(P, 1).astype(f32)
    ident = np.eye(P, dtype=f32)
    onesc = np.stack([np.ones(P, f32), np.full(P, 0.5, f32)], axis=1)
    maskr = np.zeros((1, P), f32)
    maskr[0, :64] = 1.0
    redw = np.zeros((P, 64), f32)
    redw[:, 0] = 1.0
    redw[:, 32] = 0.5

    per_core = ntiles * nt
    in_maps = []
    for c in range(NCORES):
        sl = slice(c * per_core, (c + 1) * per_core)
        xt = aug_t(T[sl])
        xl = aug_t(l[sl])
        xp = aug_t(l[sl] + l1_dot[sl])
        xm = aug_t(l[sl] - l1_dot[sl])

        def tiles(x):  # (9, per_core) -> (nd, 27, nt2) bf16 [xh; xl; xh]
            hi, lo = bsplit(x)
            st = np.concatenate([hi, lo, hi], axis=0)  # (27, per_core)
            return np.ascontiguousarray(
                st.reshape(27, nd, nt2).transpose(1, 0, 2)
            )

        xt_t, xl_t, xp_t, xm_t = tiles(xt), tiles(xl), tiles(xp), tiles(xm)
        stkA = np.ascontiguousarray(
            np.stack([xl_t, xl_t, xp_t, xp_t], axis=1)
        )  # (nd, 4, 9, nt2)
        stkB = np.ascontiguousarray(np.stack([xm_t, xm_t, xt_t, xt_t], axis=1))
        ctr_f = aug_t(center[c * ncenter:(c + 1) * ncenter])
        ch, cl = bsplit(ctr_f)
        ctr = np.ascontiguousarray(np.concatenate([ch, cl, ch], axis=0))

        in_maps.append({
            "stkA": stkA, "stkB": stkB, "wstk": wstk,
            "amat": w1_2.astype(f32), "bmat": w2_2.astype(f32),
            "wvec": wvec, "b3v": b3v, "b4v": b4v, "ident": ident,
            "onesc": onesc, "maskr": maskr, "redw": redw, "ctr": ctr,
        })
    return in_maps


_NC_CACHE = {}


def _get_nc(key=(NTILES, NT, CPER)):
    if key not in _NC_CACHE:
        _NC_CACHE[key] = _build_program(*key)
    return _NC_CACHE[key]


def _run(inputs, trace=False, tmpdir=None):
    from concourse.bass_utils import run_bass_kernel_spmd

    nc = _get_nc()
    in_maps = _prep_inputs(**inputs)
    res = run_bass_kernel_spmd(
        nc, in_maps, list(range(NCORES)), trace=trace, tmpdir=tmpdir
    )
    vt = np.concatenate([r["vt_o"].reshape(-1) for r in res.results])
    vy = np.concatenate([r["vy_o"].reshape(-1) for r in res.results])
    vg = np.concatenate([r["vg_o"].reshape(-1) for r in res.results])
    vc = np.concatenate([r["vc_o"].reshape(-1) for r in res.results])
    out = (
        vt.reshape(BATCH, 1).astype(np.float32),
        vy.reshape(BATCH, 1).astype(np.float32),
        vg.astype(np.float32),
        vc.reshape(512, 1).astype(np.float32),
    )
    return out, res


def kernel(**inputs):
    out, _ = _run(inputs, trace=False)
    return out


# revision 23
# speedup vs baseline: 1.0219x; 1.0219x over previous
"""Trainium2 Bass kernel for nn_Net_58712202936903 (dense_mlp).

Math restructuring (validated vs reference to ~5e-7 rel):
  Network: y = (W1 x + b1) * (W2 x + b2)            (layer 1, "SKIP")
           h = (A y + b3) * (B y + b4)              (layer 2, "MUL")
           v = w_out . h
  Layer2+output collapse to a quadratic form in y:
           v(y) = y^T C y + d^T y + e
      C = A^T diag(w) B,  d = B^T (w*b3) + A^T (w*b4),  e = b3 . (w*b4)
  JVP (grad term): v_grad = u1^T (S y + d),  S = C + C^T,
      u1 = z1*z2d + z2*z1d computed EXACTLY via the central difference of
      the bilinear layer-1 product: 2*u1 = y(x+xd) - y(x-xd).

Distribution: pure data parallel over 8 cores; batch 65536 -> 8192/core,
center 512 -> 64/core. Weights replicated; identical SPMD program per core.
"""

import os
import numpy as np

P = 128
NCORES = 8
BATCH = 65536
PER_CORE = BATCH // NCORES      # 8192
NT = 512                        # columns per compute tile
NTILES = PER_CORE // NT         # 16
CPER = 512 // NCORES            # center cols per core = 64

# float32r single-pass PE mode (reduced precision) for main-loop matmuls.
F32R = os.environ.get("KB_F32R", "0") == "1"


def _build_program(ntiles=NTILES, nt=NT, ncenter=CPER):
    import concourse.bass as bass
    import concourse.tile as tile
    from concourse import bacc, mybir

    f32 = mybir.dt.float32
    AF = mybir.ActivationFunctionType
    OP = mybir.AluOpType

    nc = bacc.Bacc("TRN2", target_bir_lowering=False, debug=False)
    fmm = mybir.dt.float32r if F32R else f32
    bf16 = mybir.dt.bfloat16
    KL1 = 27                    # bf16x2 K-stack: [Wh;Wh;Wl] . [xh;xl;xh]

    NDF = 2
    nd = ntiles // NDF          # DMA macro-tiles
    nt2 = NDF * nt

    # ---- DRAM parameters (per-core) ----
    stkA = nc.dram_tensor("stkA", [nd, 4, KL1, nt2], bf16, kind="ExternalInput").ap()
    stkB = nc.dram_tensor("stkB", [nd, 4, KL1, nt2], bf16, kind="ExternalInput").ap()
    wstk = nc.dram_tensor("wstk", [4, KL1, P], bf16, kind="ExternalInput").ap()
    amat = nc.dram_tensor("amat", [P, P], f32, kind="ExternalInput").ap()
    bmat = nc.dram_tensor("bmat", [P, P], f32, kind="ExternalInput").ap()
    wvec = nc.dram_tensor("wvec", [P, 1], f32, kind="ExternalInput").ap()
    b3v = nc.dram_tensor("b3v", [P, 1], f32, kind="ExternalInput").ap()
    b4v = nc.dram_tensor("b4v", [P, 1], f32, kind="ExternalInput").ap()
    ident = nc.dram_tensor("ident", [P, P], f32, kind="ExternalInput").ap()
    onesc = nc.dram_tensor("onesc", [P, 2], fmm, kind="ExternalInput").ap()
    redw = nc.dram_tensor("redw", [P, 64], fmm, kind="ExternalInput").ap()
    maskr = nc.dram_tensor("maskr", [1, P], f32, kind="ExternalInput").ap()
    ctr = nc.dram_tensor("ctr", [KL1, ncenter], bf16, kind="ExternalInput").ap()

    vt_o = nc.dram_tensor("vt_o", [1, ntiles * nt], f32, kind="ExternalOutput").ap()
    vy_o = nc.dram_tensor("vy_o", [1, ntiles * nt], f32, kind="ExternalOutput").ap()
    vg_o = nc.dram_tensor("vg_o", [1, ntiles * nt], f32, kind="ExternalOutput").ap()
    vc_o = nc.dram_tensor("vc_o", [1, ncenter], f32, kind="ExternalOutput").ap()

    with tile.TileContext(nc) as tc:
        with tc.tile_pool(name="const", bufs=1) as cpool:
          with (
            tc.tile_pool(name="prepsum", bufs=2, space="PSUM") as prepsum,
            tc.tile_pool(name="presb", bufs=1) as presb,
          ):
            # ---- constants into SBUF ----
            wst = cpool.tile([P, P], bf16)
            for g in range(4):
                nc.sync.dma_start(wst[32 * g:32 * g + KL1, :], wstk[g])
            w2low = cpool.tile([KL1, P], bf16)
            nc.sync.dma_start(w2low[:], wstk[1])
            A_sb = cpool.tile([P, P], f32)
            nc.sync.dma_start(A_sb[:], amat)
            B_sb = cpool.tile([P, P], f32)
            nc.sync.dma_start(B_sb[:], bmat)
            w_sb = cpool.tile([P, 1], f32)
            nc.sync.dma_start(w_sb[:], wvec)
            b3_sb = cpool.tile([P, 1], f32)
            nc.sync.dma_start(b3_sb[:], b3v)
            b4_sb = cpool.tile([P, 1], f32)
            nc.sync.dma_start(b4_sb[:], b4v)
            I_sb = cpool.tile([P, P], f32)
            nc.sync.dma_start(I_sb[:], ident)
            onespair_sb = cpool.tile([P, 2], fmm)
            nc.sync.dma_start(onespair_sb[:], onesc)
            ones_sb = onespair_sb[:, 0:1]
            halfones_sb = onespair_sb[:, 1:2]
            redw_sb = cpool.tile([P, 64], fmm)
            nc.sync.dma_start(redw_sb[:], redw)
            onespad = redw_sb[:, 0:32]
            halfpad = redw_sb[:, 32:64]
            maskrow_sb = cpool.tile([1, P], f32)
            nc.sync.dma_start(maskrow_sb[:], maskr)

            # ---- one-time weight transforms on device ----
            _pc = [0]

            def pre_ps(shape):
                _pc[0] += 1
                t = prepsum.tile([P, P], f32, tag="pps", name=f"pps{_pc[0]}")
                return t[: shape[0], : shape[1]]

            # wB = diag(w) @ B
            wB_sb = presb.tile([P, P], f32)
            nc.vector.scalar_tensor_tensor(
                wB_sb[:], B_sb[:], 1.0, w_sb[:].to_broadcast((P, P)), OP.mult, OP.mult
            )
            # C = A^T (diag(w) B)
            C_ps = pre_ps((P, P))
            nc.tensor.matmul(C_ps, A_sb[:], wB_sb[:], start=True, stop=True)
            C_sb = presb.tile([P, P], f32)
            nc.vector.tensor_copy(C_sb[:], C_ps)
            CT_ps = pre_ps((P, P))
            nc.tensor.transpose(CT_ps, C_sb[:], I_sb[:])
            CT_sb = cpool.tile([P, P], fmm)     # lhsT for r = C y
            nc.vector.tensor_copy(CT_sb[:], CT_ps)
            CTf_sb = presb.tile([P, P], f32)
            nc.vector.tensor_copy(CTf_sb[:], CT_ps)
            S_sb = cpool.tile([P, P], f32)      # = C + C^T (symmetric)
            nc.vector.tensor_tensor(S_sb[:], C_sb[:], CTf_sb[:], OP.add)
            # bf16x2 splits of the L2 stationaries (lhsT = C^T and S)
            Ch_sb = cpool.tile([P, P], bf16)
            nc.vector.tensor_copy(Ch_sb[:], CTf_sb[:])
            Cl_sb = cpool.tile([P, P], bf16)
            nc.vector.tensor_tensor(Cl_sb[:], CTf_sb[:], Ch_sb[:], OP.subtract)
            Sh_sb = cpool.tile([P, P], bf16)
            nc.vector.tensor_copy(Sh_sb[:], S_sb[:])
            Sl_sb = cpool.tile([P, P], bf16)
            nc.vector.tensor_tensor(Sl_sb[:], S_sb[:], Sh_sb[:], OP.subtract)
            # d = B^T (w*b3) + A^T (w*b4)
            wb3_sb = presb.tile([P, 1], f32)
            nc.vector.tensor_tensor(wb3_sb[:], b3_sb[:], w_sb[:], OP.mult)
            wb4_sb = presb.tile([P, 1], f32)
            nc.vector.tensor_tensor(wb4_sb[:], b4_sb[:], w_sb[:], OP.mult)
            d_ps = pre_ps((P, 1))
            nc.tensor.matmul(d_ps, B_sb[:], wb3_sb[:], start=True, stop=False)
            nc.tensor.matmul(d_ps, A_sb[:], wb4_sb[:], start=False, stop=True)
            d_sb = cpool.tile([P, 1], f32)
            nc.vector.tensor_copy(d_sb[:], d_ps)
            # e = b3 . (w*b4); ebias has e on partitions 0..63, 0 above
            # (so one ACT applies +e to v_t/v_y rows 0/32 and +0 to v_grad row 64)
            e_ps = pre_ps((1, 1))
            nc.tensor.matmul(e_ps, b3_sb[:], wb4_sb[:], start=True, stop=True)
            e_sb = presb.tile([1, 1], f32)
            nc.vector.tensor_copy(e_sb[:], e_ps)
            ebias_ps = pre_ps((P, 1))
            nc.tensor.matmul(ebias_ps, maskrow_sb[:], e_sb[:], start=True, stop=True)
            ebias_sb = cpool.tile([P, 1], f32)
            nc.vector.tensor_copy(ebias_sb[:], ebias_ps)

          with (
            tc.tile_pool(name="xin", bufs=4) as xpool,
            tc.tile_pool(name="zp", bufs=5, space="PSUM") as zpool,
            tc.tile_pool(name="rp", bufs=2, space="PSUM") as rpool,
            tc.tile_pool(name="vp", bufs=1, space="PSUM") as vpool,
            tc.tile_pool(name="pr", bufs=9) as ppool,
            tc.tile_pool(name="sb", bufs=3) as spool,
            tc.tile_pool(name="vout", bufs=1) as opool,
          ):
            vout = opool.tile([65, ntiles * nt], f32)
            vcout = opool.tile([1, ncenter], f32)

            HB = min(4, nd)             # DMA macro-tiles per phase block
            for blk in range(nd // HB):
                prods = {}
                # ---------- phase A: layer 1 + Hadamard products ----------
                for dj in range(HB):
                    di = blk * HB + dj
                    xa = xpool.tile([P, nt2], bf16, tag="xa")
                    xb = xpool.tile([P, nt2], bf16, tag="xb")
                    for g in range(4):
                        nc.sync.dma_start(xa[32 * g:32 * g + KL1, :], stkA[di, g])
                        nc.sync.dma_start(xb[32 * g:32 * g + KL1, :], stkB[di, g])
                    for j in range(NDF):
                        i = NDF * di + j
                        cs = slice(j * nt, (j + 1) * nt)
                        zA = []
                        for g in range(4):
                            z = zpool.tile([P, nt], f32, tag="z", name=f"zA{g}_{i}")
                            nc.tensor.matmul(
                                z[:], wst[32 * g:32 * g + KL1, :],
                                xa[32 * g:32 * g + KL1, cs],
                                start=True, stop=True, tile_position=(32 * g, 0),
                            )
                            zA.append(z)
                        c1 = spool.tile([P, nt], f32, tag="c1")
                        nc.scalar.activation(c1[:], zA[1][:], AF.Copy)
                        yl = ppool.tile([P, nt], fmm, tag="yl", name=f"yl_{i}")
                        nc.vector.tensor_tensor(yl[:], zA[0][:], c1[:], OP.mult)
                        c3 = spool.tile([P, nt], f32, tag="c3")
                        nc.scalar.activation(c3[:], zA[3][:], AF.Copy)
                        Pp = spool.tile([P, nt], f32, tag="Pp")
                        nc.vector.tensor_tensor(Pp[:], zA[2][:], c3[:], OP.mult)

                        zB = []
                        for g in range(4):
                            z = zpool.tile([P, nt], f32, tag="z", name=f"zB{g}_{i}")
                            nc.tensor.matmul(
                                z[:], wst[32 * g:32 * g + KL1, :],
                                xb[32 * g:32 * g + KL1, cs],
                                start=True, stop=True, tile_position=(32 * g, 0),
                            )
                            zB.append(z)
                        c5 = spool.tile([P, nt], f32, tag="c5")
                        nc.scalar.activation(c5[:], zB[1][:], AF.Copy)
                        c6 = spool.tile([P, nt], f32, tag="c6")
                        nc.scalar.activation(c6[:], zB[0][:], AF.Copy)
                        Mm = spool.tile([P, nt], f32, tag="Mm")
                        nc.gpsimd.tensor_tensor(Mm[:], c6[:], c5[:], OP.mult)
                        c7 = spool.tile([P, nt], f32, tag="c7")
                        nc.scalar.activation(c7[:], zB[3][:], AF.Copy)
                        yT = ppool.tile([P, nt], fmm, tag="yT", name=f"yT_{i}")
                        nc.vector.tensor_tensor(yT[:], zB[2][:], c7[:], OP.mult)

                        # u2 = Pp - Mm = 2*u1 (SBUF only -> GpSimd)
                        u2 = ppool.tile([P, nt], f32, tag="u2", name=f"u2_{i}")
                        nc.gpsimd.tensor_tensor(u2[:], Pp[:], Mm[:], OP.subtract)
                        prods[i] = (yl, yT, u2)

                # ---------- phase B: layer-2 matvecs, w, reductions ----------
                for dj in range(HB):
                    for j in range(NDF):
                        i = NDF * (blk * HB + dj) + j
                        sl = slice(i * nt, (i + 1) * nt)
                        yl, yT, u2 = prods[i]
                        rT = rpool.tile([P, nt], f32, tag="r", name=f"rT_{i}")
                        nc.tensor.matmul(rT[:], CT_sb[:], yT[:],
                                         start=True, stop=True)
                        wT = spool.tile([P, nt], fmm, tag="wT")
                        nc.vector.scalar_tensor_tensor(
                            wT[:], rT[:], d_sb[:], yT[:], OP.add, OP.mult
                        )
                        rY = rpool.tile([P, nt], f32, tag="r", name=f"rY_{i}")
                        nc.tensor.matmul(rY[:], CT_sb[:], yl[:],
                                         start=True, stop=True)
                        wY = spool.tile([P, nt], fmm, tag="wY")
                        nc.vector.scalar_tensor_tensor(
                            wY[:], rY[:], d_sb[:], yl[:], OP.add, OP.mult
                        )
                        rG = rpool.tile([P, nt], f32, tag="r", name=f"rG_{i}")
                        nc.tensor.matmul(rG[:], S_sb[:], yl[:],
                                         start=True, stop=True)
                        wG = spool.tile([P, nt], fmm, tag="wG")
                        nc.vector.scalar_tensor_tensor(
                            wG[:], rG[:], d_sb[:], u2[:], OP.add, OP.mult
                        )

                        # reductions, col-tiled into one PSUM bank
                        psV = vpool.tile([P, nt], f32, tag="v", name=f"v_{i}")
                        nc.tensor.matmul(psV[0:32, :], onespad, wT[:],
                                         start=True, stop=True,
                                         tile_position=(0, 0))
                        nc.tensor.matmul(psV[32:64, :], onespad, wY[:],
                                         start=True, stop=True,
                                         tile_position=(0, 32))
                        nc.tensor.matmul(psV[64:96, :], halfpad, wG[:],
                                         start=True, stop=True,
                                         tile_position=(0, 64))
                        nc.scalar.activation(
                            vout[0:65, sl], psV[0:65, :],
                            AF.Identity, bias=ebias_sb[0:65, :],
                        )

            # ---- center stream (tiny) ----
            xc = xpool.tile([KL1, ncenter], bf16, tag="xc")
            nc.sync.dma_start(xc[:], ctr)
            zc1 = zpool.tile([P, nt], f32, tag="z")
            zc2 = zpool.tile([P, nt], f32, tag="z")
            nc.tensor.matmul(zc1[:, :ncenter], wst[0:KL1, :], xc[:],
                             start=True, stop=True, tile_position=(0, 0))
            nc.tensor.matmul(zc2[:, :ncenter], w2low[:], xc[:],
                             start=True, stop=True, tile_position=(0, 0))
            cc = spool.tile([P, nt], f32, tag="c1")
            nc.scalar.activation(cc[:, :ncenter], zc2[:, :ncenter], AF.Copy)
            yc = spool.tile([P, nt], fmm, tag="yl")
            nc.vector.tensor_tensor(
                yc[:, :ncenter], zc1[:, :ncenter], cc[:, :ncenter], OP.mult
            )
            rc = rpool.tile([P, nt], f32, tag="r")
            nc.tensor.matmul(rc[:, :ncenter], CT_sb[:], yc[:, :ncenter],
                             start=True, stop=True)
            wc = spool.tile([P, nt], fmm, tag="wT")
            nc.vector.scalar_tensor_tensor(
                wc[:, :ncenter], rc[:, :ncenter], d_sb[:], yc[:, :ncenter],
                OP.add, OP.mult,
            )
            psC = rpool.tile([P, nt], f32, tag="r")
            nc.tensor.matmul(psC[0:1, :ncenter], ones_sb, wc[:, :ncenter],
                             start=True, stop=True)
            nc.scalar.activation(
                vcout[0:1, :], psC[0:1, :ncenter],
                AF.Identity, bias=ebias_sb[0:1, :],
            )

            # ---- output DMAs ----
            nc.sync.dma_start(vt_o, vout[0:1, :])
            nc.sync.dma_start(vy_o, vout[32:33, :])
            nc.sync.dma_start(vg_o, vout[64:65, :])
            nc.sync.dma_start(vc_o, vcout[:])

    nc.compile()
    return nc


def _prep_inputs(T, l, l1_dot, center, w1_1, b1_1, w2_1, b2_1,
                 w1_2, b1_2, w2_2, b2_2, w_out,
                 ntiles=NTILES, nt=NT, ncenter=CPER):
    """Host-side shard + layout prep. Returns list of per-core input dicts."""
    import ml_dtypes
    f32 = np.float32
    bf = ml_dtypes.bfloat16
    NDF = 2
    nd, nt2 = ntiles // NDF, NDF * nt

    def bsplit(x):  # -> (hi, lo) bf16 arrays
        hi = x.astype(bf)
        lo = (x - hi.astype(f32)).astype(bf)
        return hi, lo

    def aug_t(x):  # (n, 8) -> (9, n) with ones row
        n = x.shape[0]
        out = np.empty((9, n), f32)
        out[0:8] = x.T
        out[8] = 1.0
        return out

    W1a = np.concatenate([w1_1, b1_1[:, None]], axis=1).astype(f32)  # (128, 9)
    W2a = np.concatenate([w2_1, b2_1[:, None]], axis=1).astype(f32)

    def wstack(W):  # (128, 9) -> (27, 128) bf16 [Wh; Wh; Wl]
        hi, lo = bsplit(W.T)
        return np.concatenate([hi, hi, lo], axis=0)

    w1s, w2s = wstack(W1a), wstack(W2a)
    wstk = np.stack([w1s, w2s, w1s, w2s])  # (4, 27, 128) bf16
    wvec = w_out.reshape(P, 1).astype(f32)
    b3v = b1_2.reshape(P, 1).astype(f32)
    b4v = b2_2.reshape(P, 1).astype(f32)
    ident = np.eye(P, dtype=f32)
    onesc = np.stack([np.ones(P, f32), np.full(P, 0.5, f32)], axis=1)
    maskr = np.zeros((1, P), f32)
    maskr[0, :64] = 1.0
    redw = np.zeros((P, 64), f32)
    redw[:, 0] = 1.0
    redw[:, 32] = 0.5

    per_core = ntiles * nt
    in_maps = []
    for c in range(NCORES):
        sl = slice(c * per_core, (c + 1) * per_core)
        xt = aug_t(T[sl])
        xl = aug_t(l[sl])
        xp = aug_t(l[sl] + l1_dot[sl])
        xm = aug_t(l[sl] - l1_dot[sl])

        def tiles(x):  # (9, per_core) -> (nd, 27, nt2) bf16 [xh; xl; xh]
            hi, lo = bsplit(x)
            st = np.concatenate([hi, lo, hi], axis=0)  # (27, per_core)
            return np.ascontiguousarray(
                st.reshape(27, nd, nt2).transpose(1, 0, 2)
            )

        xt_t, xl_t, xp_t, xm_t = tiles(xt), tiles(xl), tiles(xp), tiles(xm)
        stkA = np.ascontiguousarray(
            np.stack([xl_t, xl_t, xp_t, xp_t], axis=1)
        )  # (nd, 4, 9, nt2)
        stkB = np.ascontiguousarray(np.stack([xm_t, xm_t, xt_t, xt_t], axis=1))
        ctr_f = aug_t(center[c * ncenter:(c + 1) * ncenter])
        ch, cl = bsplit(ctr_f)
        ctr = np.ascontiguousarray(np.concatenate([ch, cl, ch], axis=0))

        in_maps.append({
            "stkA": stkA, "stkB": stkB, "wstk": wstk,
            "amat": w1_2.astype(f32), "bmat": w2_2.astype(f32),
            "wvec": wvec, "b3v": b3v, "b4v": b4v, "ident": ident,
            "onesc": onesc, "maskr": maskr, "redw": redw, "ctr": ctr,
        })
    return in_maps


_NC_CACHE = {}


def _get_nc(key=(NTILES, NT, CPER)):
    if key not in _NC_CACHE:
        _NC_CACHE[key] = _build_program(*key)
    return _NC_CACHE[key]


def _run(inputs, trace=False, tmpdir=None):
    from concourse.bass_utils import run_bass_kernel_spmd

    nc = _get_nc()
    in_maps = _prep_inputs(**inputs)
    res = run_bass_kernel_spmd(
        nc, in_maps, list(range(NCORES)), trace=trace, tmpdir=tmpdir
    )
    vt = np.concatenate([r["vt_o"].reshape(-1) for r in res.results])
    vy = np.concatenate([r["vy_o"].reshape(-1) for r in res.results])
    vg = np.concatenate([r["vg_o"].reshape(-1) for r in res.results])
    vc = np.concatenate([r["vc_o"].reshape(-1) for r in res.results])
    out = (
        vt.reshape(BATCH, 1).astype(np.float32),
        vy.reshape(BATCH, 1).astype(np.float32),
        vg.astype(np.float32),
        vc.reshape(512, 1).astype(np.float32),
    )
    return out, res


def kernel(**inputs):
    inputs = {k: np.asarray(v, dtype=np.float32) for k, v in inputs.items()}
    out, _ = _run(inputs, trace=False)
    return out


# revision 25
# speedup vs baseline: 1.0282x; 1.0062x over previous
"""Trainium2 Bass kernel for nn_Net_58712202936903 (dense_mlp).

Math restructuring (validated vs reference to ~5e-7 rel):
  Network: y = (W1 x + b1) * (W2 x + b2)            (layer 1, "SKIP")
           h = (A y + b3) * (B y + b4)              (layer 2, "MUL")
           v = w_out . h
  Layer2+output collapse to a quadratic form in y:
           v(y) = y^T C y + d^T y + e
      C = A^T diag(w) B,  d = B^T (w*b3) + A^T (w*b4),  e = b3 . (w*b4)
  JVP (grad term): v_grad = u1^T (S y + d),  S = C + C^T,
      u1 = z1*z2d + z2*z1d computed EXACTLY via the central difference of
      the bilinear layer-1 product: 2*u1 = y(x+xd) - y(x-xd).

Distribution: pure data parallel over 8 cores; batch 65536 -> 8192/core,
center 512 -> 64/core. Weights replicated; identical SPMD program per core.
"""

import os
import numpy as np

P = 128
NCORES = 8
BATCH = 65536
PER_CORE = BATCH // NCORES      # 8192
NT = 512                        # columns per compute tile
NTILES = PER_CORE // NT         # 16
CPER = 512 // NCORES            # center cols per core = 64

# float32r single-pass PE mode (reduced precision) for main-loop matmuls.
F32R = os.environ.get("KB_F32R", "0") == "1"


def _build_program(ntiles=NTILES, nt=NT, ncenter=CPER):
    import concourse.bass as bass
    import concourse.tile as tile
    from concourse import bacc, mybir

    f32 = mybir.dt.float32
    AF = mybir.ActivationFunctionType
    OP = mybir.AluOpType

    nc = bacc.Bacc("TRN2", target_bir_lowering=False, debug=False)
    fmm = mybir.dt.float32r if F32R else f32
    bf16 = mybir.dt.bfloat16
    KL1 = 27                    # bf16x2 K-stack: [Wh;Wh;Wl] . [xh;xl;xh]

    NDF = 2
    nd = ntiles // NDF          # DMA macro-tiles
    nt2 = NDF * nt

    # ---- DRAM parameters (per-core) ----
    stkA = nc.dram_tensor("stkA", [nd, 4, KL1, nt2], bf16, kind="ExternalInput").ap()
    stkB = nc.dram_tensor("stkB", [nd, 4, KL1, nt2], bf16, kind="ExternalInput").ap()
    wstk = nc.dram_tensor("wstk", [4, KL1, P], bf16, kind="ExternalInput").ap()
    amat = nc.dram_tensor("amat", [P, P], f32, kind="ExternalInput").ap()
    bmat = nc.dram_tensor("bmat", [P, P], f32, kind="ExternalInput").ap()
    wvec = nc.dram_tensor("wvec", [P, 1], f32, kind="ExternalInput").ap()
    b3v = nc.dram_tensor("b3v", [P, 1], f32, kind="ExternalInput").ap()
    b4v = nc.dram_tensor("b4v", [P, 1], f32, kind="ExternalInput").ap()
    ident = nc.dram_tensor("ident", [P, P], f32, kind="ExternalInput").ap()
    onesc = nc.dram_tensor("onesc", [P, 2], fmm, kind="ExternalInput").ap()
    redw = nc.dram_tensor("redw", [P, 64], fmm, kind="ExternalInput").ap()
    maskr = nc.dram_tensor("maskr", [1, P], f32, kind="ExternalInput").ap()
    ctr = nc.dram_tensor("ctr", [KL1, ncenter], bf16, kind="ExternalInput").ap()

    vt_o = nc.dram_tensor("vt_o", [1, ntiles * nt], f32, kind="ExternalOutput").ap()
    vy_o = nc.dram_tensor("vy_o", [1, ntiles * nt], f32, kind="ExternalOutput").ap()
    vg_o = nc.dram_tensor("vg_o", [1, ntiles * nt], f32, kind="ExternalOutput").ap()
    vc_o = nc.dram_tensor("vc_o", [1, ncenter], f32, kind="ExternalOutput").ap()

    with tile.TileContext(nc) as tc:
        with tc.tile_pool(name="const", bufs=1) as cpool:
          with (
            tc.tile_pool(name="prepsum", bufs=2, space="PSUM") as prepsum,
            tc.tile_pool(name="presb", bufs=1) as presb,
          ):
            # ---- constants into SBUF ----
            wst = cpool.tile([P, P], bf16)
            for g in range(4):
                nc.sync.dma_start(wst[32 * g:32 * g + KL1, :], wstk[g])
            w2low = cpool.tile([KL1, P], bf16)
            nc.sync.dma_start(w2low[:], wstk[1])
            A_sb = cpool.tile([P, P], f32)
            nc.sync.dma_start(A_sb[:], amat)
            B_sb = cpool.tile([P, P], f32)
            nc.sync.dma_start(B_sb[:], bmat)
            w_sb = cpool.tile([P, 1], f32)
            nc.sync.dma_start(w_sb[:], wvec)
            b3_sb = cpool.tile([P, 1], f32)
            nc.sync.dma_start(b3_sb[:], b3v)
            b4_sb = cpool.tile([P, 1], f32)
            nc.sync.dma_start(b4_sb[:], b4v)
            I_sb = cpool.tile([P, P], f32)
            nc.sync.dma_start(I_sb[:], ident)
            onespair_sb = cpool.tile([P, 2], fmm)
            nc.sync.dma_start(onespair_sb[:], onesc)
            ones_sb = onespair_sb[:, 0:1]
            halfones_sb = onespair_sb[:, 1:2]
            redw_sb = cpool.tile([P, 64], fmm)
            nc.sync.dma_start(redw_sb[:], redw)
            onespad = redw_sb[:, 0:32]
            halfpad = redw_sb[:, 32:64]
            maskrow_sb = cpool.tile([1, P], f32)
            nc.sync.dma_start(maskrow_sb[:], maskr)

            # ---- one-time weight transforms on device ----
            _pc = [0]

            def pre_ps(shape):
                _pc[0] += 1
                t = prepsum.tile([P, P], f32, tag="pps", name=f"pps{_pc[0]}")
                return t[: shape[0], : shape[1]]

            # wB = diag(w) @ B
            wB_sb = presb.tile([P, P], f32)
            nc.vector.scalar_tensor_tensor(
                wB_sb[:], B_sb[:], 1.0, w_sb[:].to_broadcast((P, P)), OP.mult, OP.mult
            )
            # C = A^T (diag(w) B)
            C_ps = pre_ps((P, P))
            nc.tensor.matmul(C_ps, A_sb[:], wB_sb[:], start=True, stop=True)
            C_sb = presb.tile([P, P], f32)
            nc.vector.tensor_copy(C_sb[:], C_ps)
            CT_ps = pre_ps((P, P))
            nc.tensor.transpose(CT_ps, C_sb[:], I_sb[:])
            CT_sb = cpool.tile([P, P], fmm)     # lhsT for r = C y
            nc.vector.tensor_copy(CT_sb[:], CT_ps)
            CTf_sb = presb.tile([P, P], f32)
            nc.vector.tensor_copy(CTf_sb[:], CT_ps)
            S_sb = cpool.tile([P, P], f32)      # = C + C^T (symmetric)
            nc.vector.tensor_tensor(S_sb[:], C_sb[:], CTf_sb[:], OP.add)
            # bf16x2 splits of the L2 stationaries (lhsT = C^T and S)
            Ch_sb = cpool.tile([P, P], bf16)
            nc.vector.tensor_copy(Ch_sb[:], CTf_sb[:])
            Cl_sb = cpool.tile([P, P], bf16)
            nc.vector.tensor_tensor(Cl_sb[:], CTf_sb[:], Ch_sb[:], OP.subtract)
            Sh_sb = cpool.tile([P, P], bf16)
            nc.vector.tensor_copy(Sh_sb[:], S_sb[:])
            Sl_sb = cpool.tile([P, P], bf16)
            nc.vector.tensor_tensor(Sl_sb[:], S_sb[:], Sh_sb[:], OP.subtract)
            # d = B^T (w*b3) + A^T (w*b4)
            wb3_sb = presb.tile([P, 1], f32)
            nc.vector.tensor_tensor(wb3_sb[:], b3_sb[:], w_sb[:], OP.mult)
            wb4_sb = presb.tile([P, 1], f32)
            nc.vector.tensor_tensor(wb4_sb[:], b4_sb[:], w_sb[:], OP.mult)
            d_ps = pre_ps((P, 1))
            nc.tensor.matmul(d_ps, B_sb[:], wb3_sb[:], start=True, stop=False)
            nc.tensor.matmul(d_ps, A_sb[:], wb4_sb[:], start=False, stop=True)
            d_sb = cpool.tile([P, 1], f32)
            nc.vector.tensor_copy(d_sb[:], d_ps)
            # e = b3 . (w*b4); ebias has e on partitions 0..63, 0 above
            # (so one ACT applies +e to v_t/v_y rows 0/32 and +0 to v_grad row 64)
            e_ps = pre_ps((1, 1))
            nc.tensor.matmul(e_ps, b3_sb[:], wb4_sb[:], start=True, stop=True)
            e_sb = presb.tile([1, 1], f32)
            nc.vector.tensor_copy(e_sb[:], e_ps)
            ebias_ps = pre_ps((P, 1))
            nc.tensor.matmul(ebias_ps, maskrow_sb[:], e_sb[:], start=True, stop=True)
            ebias_sb = cpool.tile([P, 1], f32)
            nc.vector.tensor_copy(ebias_sb[:], ebias_ps)

          with (
            tc.tile_pool(name="xin", bufs=4) as xpool,
            tc.tile_pool(name="zp", bufs=5, space="PSUM") as zpool,
            tc.tile_pool(name="rp", bufs=2, space="PSUM") as rpool,
            tc.tile_pool(name="vp", bufs=1, space="PSUM") as vpool,
            tc.tile_pool(name="pr", bufs=9) as ppool,
            tc.tile_pool(name="sb", bufs=3) as spool,
            tc.tile_pool(name="vout", bufs=1) as opool,
          ):
            vout = opool.tile([65, ntiles * nt], f32)
            vcout = opool.tile([1, ncenter], f32)

            # ---- center stream (tiny) ----
            xc = xpool.tile([KL1, ncenter], bf16, tag="xc")
            nc.sync.dma_start(xc[:], ctr)
            zc1 = zpool.tile([P, nt], f32, tag="z")
            zc2 = zpool.tile([P, nt], f32, tag="z")
            nc.tensor.matmul(zc1[:, :ncenter], wst[0:KL1, :], xc[:],
                             start=True, stop=True, tile_position=(0, 0))
            nc.tensor.matmul(zc2[:, :ncenter], w2low[:], xc[:],
                             start=True, stop=True, tile_position=(0, 0))
            cc = spool.tile([P, nt], f32, tag="c1")
            nc.scalar.activation(cc[:, :ncenter], zc2[:, :ncenter], AF.Copy)
            yc = spool.tile([P, nt], fmm, tag="yl")
            nc.vector.tensor_tensor(
                yc[:, :ncenter], zc1[:, :ncenter], cc[:, :ncenter], OP.mult
            )
            rc = rpool.tile([P, nt], f32, tag="r")
            nc.tensor.matmul(rc[:, :ncenter], CT_sb[:], yc[:, :ncenter],
                             start=True, stop=True)
            wc = spool.tile([P, nt], fmm, tag="wT")
            nc.vector.scalar_tensor_tensor(
                wc[:, :ncenter], rc[:, :ncenter], d_sb[:], yc[:, :ncenter],
                OP.add, OP.mult,
            )
            psC = rpool.tile([P, nt], f32, tag="r")
            nc.tensor.matmul(psC[0:1, :ncenter], ones_sb, wc[:, :ncenter],
                             start=True, stop=True)
            nc.scalar.activation(
                vcout[0:1, :], psC[0:1, :ncenter],
                AF.Identity, bias=ebias_sb[0:1, :],
            )



            HB = min(4, nd)             # DMA macro-tiles per phase block
            for blk in range(nd // HB):
                prods = {}
                # ---------- phase A: layer 1 + Hadamard products ----------
                for dj in range(HB):
                    di = blk * HB + dj
                    xa = xpool.tile([P, nt2], bf16, tag="xa")
                    xb = xpool.tile([P, nt2], bf16, tag="xb")
                    for g in range(4):
                        nc.sync.dma_start(xa[32 * g:32 * g + KL1, :], stkA[di, g])
                        nc.sync.dma_start(xb[32 * g:32 * g + KL1, :], stkB[di, g])
                    for j in range(NDF):
                        i = NDF * di + j
                        cs = slice(j * nt, (j + 1) * nt)
                        zA = []
                        for g in range(4):
                            z = zpool.tile([P, nt], f32, tag="z", name=f"zA{g}_{i}")
                            nc.tensor.matmul(
                                z[:], wst[32 * g:32 * g + KL1, :],
                                xa[32 * g:32 * g + KL1, cs],
                                start=True, stop=True, tile_position=(32 * g, 0),
                            )
                            zA.append(z)
                        c1 = spool.tile([P, nt], f32, tag="c1")
                        nc.scalar.activation(c1[:], zA[1][:], AF.Copy)
                        yl = ppool.tile([P, nt], fmm, tag="yl", name=f"yl_{i}")
                        nc.vector.tensor_tensor(yl[:], zA[0][:], c1[:], OP.mult)
                        c3 = spool.tile([P, nt], f32, tag="c3")
                        nc.scalar.activation(c3[:], zA[3][:], AF.Copy)
                        Pp = spool.tile([P, nt], f32, tag="Pp")
                        nc.vector.tensor_tensor(Pp[:], zA[2][:], c3[:], OP.mult)

                        zB = []
                        for g in range(4):
                            z = zpool.tile([P, nt], f32, tag="z", name=f"zB{g}_{i}")
                            nc.tensor.matmul(
                                z[:], wst[32 * g:32 * g + KL1, :],
                                xb[32 * g:32 * g + KL1, cs],
                                start=True, stop=True, tile_position=(32 * g, 0),
                            )
                            zB.append(z)
                        c5 = spool.tile([P, nt], f32, tag="c5")
                        nc.scalar.activation(c5[:], zB[1][:], AF.Copy)
                        c6 = spool.tile([P, nt], f32, tag="c6")
                        nc.scalar.activation(c6[:], zB[0][:], AF.Copy)
                        Mm = spool.tile([P, nt], f32, tag="Mm")
                        nc.gpsimd.tensor_tensor(Mm[:], c6[:], c5[:], OP.mult)
                        c7 = spool.tile([P, nt], f32, tag="c7")
                        nc.scalar.activation(c7[:], zB[3][:], AF.Copy)
                        yT = ppool.tile([P, nt], fmm, tag="yT", name=f"yT_{i}")
                        nc.vector.tensor_tensor(yT[:], zB[2][:], c7[:], OP.mult)

                        # u2 = Pp - Mm = 2*u1 (SBUF only -> GpSimd)
                        u2 = ppool.tile([P, nt], f32, tag="u2", name=f"u2_{i}")
                        nc.gpsimd.tensor_tensor(u2[:], Pp[:], Mm[:], OP.subtract)
                        prods[i] = (yl, yT, u2)

                # ---------- phase B: layer-2 matvecs, w, reductions ----------
                for dj in range(HB):
                    for j in range(NDF):
                        i = NDF * (blk * HB + dj) + j
                        sl = slice(i * nt, (i + 1) * nt)
                        yl, yT, u2 = prods[i]
                        rT = rpool.tile([P, nt], f32, tag="r", name=f"rT_{i}")
                        nc.tensor.matmul(rT[:], CT_sb[:], yT[:],
                                         start=True, stop=True)
                        wT = spool.tile([P, nt], fmm, tag="wT")
                        nc.vector.scalar_tensor_tensor(
                            wT[:], rT[:], d_sb[:], yT[:], OP.add, OP.mult
                        )
                        rY = rpool.tile([P, nt], f32, tag="r", name=f"rY_{i}")
                        nc.tensor.matmul(rY[:], CT_sb[:], yl[:],
                                         start=True, stop=True)
                        wY = spool.tile([P, nt], fmm, tag="wY")
                        nc.vector.scalar_tensor_tensor(
                            wY[:], rY[:], d_sb[:], yl[:], OP.add, OP.mult
                        )
                        rG = rpool.tile([P, nt], f32, tag="r", name=f"rG_{i}")
                        nc.tensor.matmul(rG[:], S_sb[:], yl[:],
                                         start=True, stop=True)
                        wG = spool.tile([P, nt], fmm, tag="wG")
                        nc.vector.scalar_tensor_tensor(
                            wG[:], rG[:], d_sb[:], u2[:], OP.add, OP.mult
                        )

                        # reductions, col-tiled into one PSUM bank
                        psV = vpool.tile([P, nt], f32, tag="v", name=f"v_{i}")
                        nc.tensor.matmul(psV[0:32, :], onespad, wT[:],
                                         start=True, stop=True,
                                         tile_position=(0, 0))
                        nc.tensor.matmul(psV[32:64, :], onespad, wY[:],
                                         start=True, stop=True,
                                         tile_position=(0, 32))
                        nc.tensor.matmul(psV[64:96, :], halfpad, wG[:],
                                         start=True, stop=True,
                                         tile_position=(0, 64))
                        nc.scalar.activation(
                            vout[0:65, sl], psV[0:65, :],
                            AF.Identity, bias=ebias_sb[0:65, :],
                        )

                # stream this block's outputs while later blocks compute
                bsl = slice(blk * HB * NDF * nt, (blk + 1) * HB * NDF * nt)
                nc.sync.dma_start(vt_o[:, bsl], vout[0:1, bsl])
                nc.sync.dma_start(vy_o[:, bsl], vout[32:33, bsl])
                nc.sync.dma_start(vg_o[:, bsl], vout[64:65, bsl])

            # ---- center output DMA (main outputs stream per block) ----
            nc.sync.dma_start(vc_o, vcout[:])

    nc.compile()
    return nc


def _prep_inputs(T, l, l1_dot, center, w1_1, b1_1, w2_1, b2_1,
                 w1_2, b1_2, w2_2, b2_2, w_out,
                 ntiles=NTILES, nt=NT, ncenter=CPER):
    """Host-side shard + layout prep. Returns list of per-core input dicts."""
    import ml_dtypes
    f32 = np.float32
    bf = ml_dtypes.bfloat16
    NDF = 2
    nd, nt2 = ntiles // NDF, NDF * nt

    def bsplit(x):  # -> (hi, lo) bf16 arrays
        hi = x.astype(bf)
        lo = (x - hi.astype(f32)).astype(bf)
        return hi, lo

    def aug_t(x):  # (n, 8) -> (9, n) with ones row
        n = x.shape[0]
        out = np.empty((9, n), f32)
        out[0:8] = x.T
        out[8] = 1.0
        return out

    W1a = np.concatenate([w1_1, b1_1[:, None]], axis=1).astype(f32)  # (128, 9)
    W2a = np.concatenate([w2_1, b2_1[:, None]], axis=1).astype(f32)

    def wstack(W):  # (128, 9) -> (27, 128) bf16 [Wh; Wh; Wl]
        hi, lo = bsplit(W.T)
        return np.concatenate([hi, hi, lo], axis=0)

    w1s, w2s = wstack(W1a), wstack(W2a)
    wstk = np.stack([w1s, w2s, w1s, w2s])  # (4, 27, 128) bf16
    wvec = w_out.reshape(P, 1).astype(f32)
    b3v = b1_2.reshape(P, 1).astype(f32)
    b4v = b2_2.reshape(P, 1).astype(f32)
    ident = np.eye(P, dtype=f32)
    onesc = np.stack([np.ones(P, f32), np.full(P, 0.5, f32)], axis=1)
    maskr = np.zeros((1, P), f32)
    maskr[0, :64] = 1.0
    redw = np.zeros((P, 64), f32)
    redw[:, 0] = 1.0
    redw[:, 32] = 0.5

    per_core = ntiles * nt
    in_maps = []
    for c in range(NCORES):
        sl = slice(c * per_core, (c + 1) * per_core)
        xt = aug_t(T[sl])
        xl = aug_t(l[sl])
        xp = aug_t(l[sl] + l1_dot[sl])
        xm = aug_t(l[sl] - l1_dot[sl])

        def tiles(x):  # (9, per_core) -> (nd, 27, nt2) bf16 [xh; xl; xh]
            hi, lo = bsplit(x)
            st = np.concatenate([hi, lo, hi], axis=0)  # (27, per_core)
            return np.ascontiguousarray(
                st.reshape(27, nd, nt2).transpose(1, 0, 2)
            )

        xt_t, xl_t, xp_t, xm_t = tiles(xt), tiles(xl), tiles(xp), tiles(xm)
        stkA = np.ascontiguousarray(
            np.stack([xl_t, xl_t, xp_t, xp_t], axis=1)
        )  # (nd, 4, 9, nt2)
        stkB = np.ascontiguousarray(np.stack([xm_t, xm_t, xt_t, xt_t], axis=1))
        ctr_f = aug_t(center[c * ncenter:(c + 1) * ncenter])
        ch, cl = bsplit(ctr_f)
        ctr = np.ascontiguousarray(np.concatenate([ch, cl, ch], axis=0))

        in_maps.append({
            "stkA": stkA, "stkB": stkB, "wstk": wstk,
            "amat": w1_2.astype(f32), "bmat": w2_2.astype(f32),
            "wvec": wvec, "b3v": b3v, "b4v": b4v, "ident": ident,
            "onesc": onesc, "maskr": maskr, "redw": redw, "ctr": ctr,
        })
    return in_maps


_NC_CACHE = {}


def _get_nc(key=(NTILES, NT, CPER)):
    if key not in _NC_CACHE:
        _NC_CACHE[key] = _build_program(*key)
    return _NC_CACHE[key]


def _run(inputs, trace=False, tmpdir=None):
    from concourse.bass_utils import run_bass_kernel_spmd

    nc = _get_nc()
    in_maps = _prep_inputs(**inputs)
    res = run_bass_kernel_spmd(
        nc, in_maps, list(range(NCORES)), trace=trace, tmpdir=tmpdir
    )
    vt = np.concatenate([r["vt_o"].reshape(-1) for r in res.results])
    vy = np.concatenate([r["vy_o"].reshape(-1) for r in res.results])
    vg = np.concatenate([r["vg_o"].reshape(-1) for r in res.results])
    vc = np.concatenate([r["vc_o"].reshape(-1) for r in res.results])
    out = (
        vt.reshape(BATCH, 1).astype(np.float32),
        vy.reshape(BATCH, 1).astype(np.float32),
        vg.astype(np.float32),
        vc.reshape(512, 1).astype(np.float32),
    )
    return out, res


def kernel(**inputs):
    inputs = {k: np.asarray(v, dtype=np.float32) for k, v in inputs.items()}
    out, _ = _run(inputs, trace=False)
    return out


# revision 27
# speedup vs baseline: 1.0804x; 1.0508x over previous
"""Trainium2 Bass kernel for nn_Net_58712202936903 (dense_mlp).

Math restructuring (validated vs reference to ~5e-7 rel):
  Network: y = (W1 x + b1) * (W2 x + b2)            (layer 1, "SKIP")
           h = (A y + b3) * (B y + b4)              (layer 2, "MUL")
           v = w_out . h
  Layer2+output collapse to a quadratic form in y:
           v(y) = y^T C y + d^T y + e
      C = A^T diag(w) B,  d = B^T (w*b3) + A^T (w*b4),  e = b3 . (w*b4)
  JVP (grad term): v_grad = u1^T (S y + d),  S = C + C^T,
      u1 = z1*z2d + z2*z1d computed EXACTLY via the central difference of
      the bilinear layer-1 product: 2*u1 = y(x+xd) - y(x-xd).

Distribution: pure data parallel over 8 cores; batch 65536 -> 8192/core,
center 512 -> 64/core. Weights replicated; identical SPMD program per core.
"""

import os
import numpy as np

P = 128
NCORES = 8
BATCH = 65536
PER_CORE = BATCH // NCORES      # 8192
NT = 512                        # columns per compute tile
NTILES = PER_CORE // NT         # 16
CPER = 512 // NCORES            # center cols per core = 64

# float32r single-pass PE mode (reduced precision) for main-loop matmuls.
F32R = os.environ.get("KB_F32R", "0") == "1"


def _build_program(ntiles=NTILES, nt=NT, ncenter=CPER):
    import concourse.bass as bass
    import concourse.tile as tile
    from concourse import bacc, mybir

    f32 = mybir.dt.float32
    AF = mybir.ActivationFunctionType
    OP = mybir.AluOpType

    nc = bacc.Bacc("TRN2", target_bir_lowering=False, debug=False)
    fmm = mybir.dt.float32r if F32R else f32
    bf16 = mybir.dt.bfloat16
    KL1 = 27                    # bf16x2 K-stack: [Wh;Wh;Wl] . [xh;xl;xh]

    NDF = 2
    nd = ntiles // NDF          # DMA macro-tiles
    nt2 = NDF * nt

    # ---- DRAM parameters (per-core) ----
    stkA = nc.dram_tensor("stkA", [nd, 4, KL1, nt2], bf16, kind="ExternalInput").ap()
    stkB = nc.dram_tensor("stkB", [nd, 4, KL1, nt2], bf16, kind="ExternalInput").ap()
    wstk = nc.dram_tensor("wstk", [4, KL1, P], bf16, kind="ExternalInput").ap()
    amat = nc.dram_tensor("amat", [P, P], f32, kind="ExternalInput").ap()
    bmat = nc.dram_tensor("bmat", [P, P], f32, kind="ExternalInput").ap()
    wvec = nc.dram_tensor("wvec", [P, 1], f32, kind="ExternalInput").ap()
    b3v = nc.dram_tensor("b3v", [P, 1], f32, kind="ExternalInput").ap()
    b4v = nc.dram_tensor("b4v", [P, 1], f32, kind="ExternalInput").ap()
    ident = nc.dram_tensor("ident", [P, P], f32, kind="ExternalInput").ap()
    onesc = nc.dram_tensor("onesc", [P, 2], fmm, kind="ExternalInput").ap()
    redw = nc.dram_tensor("redw", [P, 64], fmm, kind="ExternalInput").ap()
    maskr = nc.dram_tensor("maskr", [1, P], f32, kind="ExternalInput").ap()
    ctr = nc.dram_tensor("ctr", [KL1, ncenter], bf16, kind="ExternalInput").ap()

    vt_o = nc.dram_tensor("vt_o", [1, ntiles * nt], f32, kind="ExternalOutput").ap()
    vy_o = nc.dram_tensor("vy_o", [1, ntiles * nt], f32, kind="ExternalOutput").ap()
    vg_o = nc.dram_tensor("vg_o", [1, ntiles * nt], f32, kind="ExternalOutput").ap()
    vc_o = nc.dram_tensor("vc_o", [1, ncenter], f32, kind="ExternalOutput").ap()

    with tile.TileContext(nc) as tc:
        with tc.tile_pool(name="const", bufs=1) as cpool:
          with (
            tc.tile_pool(name="prepsum", bufs=2, space="PSUM") as prepsum,
            tc.tile_pool(name="presb", bufs=1) as presb,
          ):
            # ---- constants into SBUF ----
            wst = cpool.tile([P, P], bf16)
            for g in range(4):
                nc.sync.dma_start(wst[32 * g:32 * g + KL1, :], wstk[g])
            w2low = cpool.tile([KL1, P], bf16)
            nc.gpsimd.dma_start(w2low[:], wstk[1])
            A_sb = cpool.tile([P, P], f32)
            nc.gpsimd.dma_start(A_sb[:], amat)
            B_sb = cpool.tile([P, P], f32)
            nc.gpsimd.dma_start(B_sb[:], bmat)
            w_sb = cpool.tile([P, 1], f32)
            nc.scalar.dma_start(w_sb[:], wvec)
            b3_sb = cpool.tile([P, 1], f32)
            nc.scalar.dma_start(b3_sb[:], b3v)
            b4_sb = cpool.tile([P, 1], f32)
            nc.scalar.dma_start(b4_sb[:], b4v)
            I_sb = cpool.tile([P, P], f32)
            nc.gpsimd.dma_start(I_sb[:], ident)
            onespair_sb = cpool.tile([P, 2], fmm)
            nc.scalar.dma_start(onespair_sb[:], onesc)
            ones_sb = onespair_sb[:, 0:1]
            halfones_sb = onespair_sb[:, 1:2]
            redw_sb = cpool.tile([P, 64], fmm)
            nc.scalar.dma_start(redw_sb[:], redw)
            onespad = redw_sb[:, 0:32]
            halfpad = redw_sb[:, 32:64]
            maskrow_sb = cpool.tile([1, P], f32)
            nc.gpsimd.dma_start(maskrow_sb[:], maskr)

            # ---- one-time weight transforms on device ----
            _pc = [0]

            def pre_ps(shape):
                _pc[0] += 1
                t = prepsum.tile([P, P], f32, tag="pps", name=f"pps{_pc[0]}")
                return t[: shape[0], : shape[1]]

            # wB = diag(w) @ B
            wB_sb = presb.tile([P, P], f32)
            nc.vector.scalar_tensor_tensor(
                wB_sb[:], B_sb[:], 1.0, w_sb[:].to_broadcast((P, P)), OP.mult, OP.mult
            )
            # C = A^T (diag(w) B)
            C_ps = pre_ps((P, P))
            nc.tensor.matmul(C_ps, A_sb[:], wB_sb[:], start=True, stop=True)
            C_sb = presb.tile([P, P], f32)
            nc.vector.tensor_copy(C_sb[:], C_ps)
            CT_ps = pre_ps((P, P))
            nc.tensor.transpose(CT_ps, C_sb[:], I_sb[:])
            CT_sb = cpool.tile([P, P], fmm)     # lhsT for r = C y
            nc.vector.tensor_copy(CT_sb[:], CT_ps)
            CTf_sb = presb.tile([P, P], f32)
            nc.vector.tensor_copy(CTf_sb[:], CT_ps)
            S_sb = cpool.tile([P, P], f32)      # = C + C^T (symmetric)
            nc.vector.tensor_tensor(S_sb[:], C_sb[:], CTf_sb[:], OP.add)
            # bf16x2 splits of the L2 stationaries (lhsT = C^T and S)
            Ch_sb = cpool.tile([P, P], bf16)
            nc.vector.tensor_copy(Ch_sb[:], CTf_sb[:])
            Cl_sb = cpool.tile([P, P], bf16)
            nc.vector.tensor_tensor(Cl_sb[:], CTf_sb[:], Ch_sb[:], OP.subtract)
            Sh_sb = cpool.tile([P, P], bf16)
            nc.vector.tensor_copy(Sh_sb[:], S_sb[:])
            Sl_sb = cpool.tile([P, P], bf16)
            nc.vector.tensor_tensor(Sl_sb[:], S_sb[:], Sh_sb[:], OP.subtract)
            # d = B^T (w*b3) + A^T (w*b4)
            wb3_sb = presb.tile([P, 1], f32)
            nc.vector.tensor_tensor(wb3_sb[:], b3_sb[:], w_sb[:], OP.mult)
            wb4_sb = presb.tile([P, 1], f32)
            nc.vector.tensor_tensor(wb4_sb[:], b4_sb[:], w_sb[:], OP.mult)
            d_ps = pre_ps((P, 1))
            nc.tensor.matmul(d_ps, B_sb[:], wb3_sb[:], start=True, stop=False)
            nc.tensor.matmul(d_ps, A_sb[:], wb4_sb[:], start=False, stop=True)
            d_sb = cpool.tile([P, 1], f32)
            nc.vector.tensor_copy(d_sb[:], d_ps)
            # e = b3 . (w*b4); ebias has e on partitions 0..63, 0 above
            # (so one ACT applies +e to v_t/v_y rows 0/32 and +0 to v_grad row 64)
            e_ps = pre_ps((1, 1))
            nc.tensor.matmul(e_ps, b3_sb[:], wb4_sb[:], start=True, stop=True)
            e_sb = presb.tile([1, 1], f32)
            nc.vector.tensor_copy(e_sb[:], e_ps)
            ebias_ps = pre_ps((P, 1))
            nc.tensor.matmul(ebias_ps, maskrow_sb[:], e_sb[:], start=True, stop=True)
            ebias_sb = cpool.tile([P, 1], f32)
            nc.vector.tensor_copy(ebias_sb[:], ebias_ps)

          with (
            tc.tile_pool(name="xin", bufs=4) as xpool,
            tc.tile_pool(name="zp", bufs=5, space="PSUM") as zpool,
            tc.tile_pool(name="rp", bufs=2, space="PSUM") as rpool,
            tc.tile_pool(name="vp", bufs=1, space="PSUM") as vpool,
            tc.tile_pool(name="pr", bufs=9) as ppool,
            tc.tile_pool(name="sb", bufs=3) as spool,
            tc.tile_pool(name="vout", bufs=1) as opool,
          ):
            vout = opool.tile([65, ntiles * nt], f32)
            vcout = opool.tile([1, ncenter], f32)

            # ---- center stream (tiny) ----
            xc = xpool.tile([KL1, ncenter], bf16, tag="xc")
            nc.scalar.dma_start(xc[:], ctr)
            zc1 = zpool.tile([P, nt], f32, tag="z")
            zc2 = zpool.tile([P, nt], f32, tag="z")
            nc.tensor.matmul(zc1[:, :ncenter], wst[0:KL1, :], xc[:],
                             start=True, stop=True, tile_position=(0, 0))
            nc.tensor.matmul(zc2[:, :ncenter], w2low[:], xc[:],
                             start=True, stop=True, tile_position=(0, 0))
            cc = spool.tile([P, nt], f32, tag="c1")
            nc.scalar.activation(cc[:, :ncenter], zc2[:, :ncenter], AF.Copy)
            yc = spool.tile([P, nt], fmm, tag="yl")
            nc.vector.tensor_tensor(
                yc[:, :ncenter], zc1[:, :ncenter], cc[:, :ncenter], OP.mult
            )
            rc = rpool.tile([P, nt], f32, tag="r")
            nc.tensor.matmul(rc[:, :ncenter], CT_sb[:], yc[:, :ncenter],
                             start=True, stop=True)
            wc = spool.tile([P, nt], fmm, tag="wT")
            nc.vector.scalar_tensor_tensor(
                wc[:, :ncenter], rc[:, :ncenter], d_sb[:], yc[:, :ncenter],
                OP.add, OP.mult,
            )
            psC = rpool.tile([P, nt], f32, tag="r")
            nc.tensor.matmul(psC[0:1, :ncenter], ones_sb, wc[:, :ncenter],
                             start=True, stop=True)
            nc.scalar.activation(
                vcout[0:1, :], psC[0:1, :ncenter],
                AF.Identity, bias=ebias_sb[0:1, :],
            )



            HB = min(4, nd)             # DMA macro-tiles per phase block
            for blk in range(nd // HB):
                prods = {}
                # ---------- phase A: layer 1 + Hadamard products ----------
                for dj in range(HB):
                    di = blk * HB + dj
                    xa = xpool.tile([P, nt2], bf16, tag="xa")
                    xb = xpool.tile([P, nt2], bf16, tag="xb")
                    if di == 0:
                        engs = [nc.sync, nc.scalar, nc.gpsimd, nc.sync]
                        for g in range(4):
                            engs[g].dma_start(xa[32 * g:32 * g + KL1, :],
                                              stkA[di, g])
                            engs[(g + 1) % 4].dma_start(
                                xb[32 * g:32 * g + KL1, :], stkB[di, g])
                    else:
                        for g in range(4):
                            nc.sync.dma_start(xa[32 * g:32 * g + KL1, :],
                                              stkA[di, g])
                            nc.sync.dma_start(xb[32 * g:32 * g + KL1, :],
                                              stkB[di, g])
                    for j in range(NDF):
                        i = NDF * di + j
                        cs = slice(j * nt, (j + 1) * nt)
                        zA = []
                        for g in range(4):
                            z = zpool.tile([P, nt], f32, tag="z", name=f"zA{g}_{i}")
                            nc.tensor.matmul(
                                z[:], wst[32 * g:32 * g + KL1, :],
                                xa[32 * g:32 * g + KL1, cs],
                                start=True, stop=True, tile_position=(32 * g, 0),
                            )
                            zA.append(z)
                        c1 = spool.tile([P, nt], f32, tag="c1")
                        nc.scalar.activation(c1[:], zA[1][:], AF.Copy)
                        yl = ppool.tile([P, nt], fmm, tag="yl", name=f"yl_{i}")
                        nc.vector.tensor_tensor(yl[:], zA[0][:], c1[:], OP.mult)
                        c3 = spool.tile([P, nt], f32, tag="c3")
                        nc.scalar.activation(c3[:], zA[3][:], AF.Copy)
                        Pp = spool.tile([P, nt], f32, tag="Pp")
                        nc.vector.tensor_tensor(Pp[:], zA[2][:], c3[:], OP.mult)

                        zB = []
                        for g in range(4):
                            z = zpool.tile([P, nt], f32, tag="z", name=f"zB{g}_{i}")
                            nc.tensor.matmul(
                                z[:], wst[32 * g:32 * g + KL1, :],
                                xb[32 * g:32 * g + KL1, cs],
                                start=True, stop=True, tile_position=(32 * g, 0),
                            )
                            zB.append(z)
                        c5 = spool.tile([P, nt], f32, tag="c5")
                        nc.scalar.activation(c5[:], zB[1][:], AF.Copy)
                        c6 = spool.tile([P, nt], f32, tag="c6")
                        nc.scalar.activation(c6[:], zB[0][:], AF.Copy)
                        Mm = spool.tile([P, nt], f32, tag="Mm")
                        nc.gpsimd.tensor_tensor(Mm[:], c6[:], c5[:], OP.mult)
                        c7 = spool.tile([P, nt], f32, tag="c7")
                        nc.scalar.activation(c7[:], zB[3][:], AF.Copy)
                        yT = ppool.tile([P, nt], fmm, tag="yT", name=f"yT_{i}")
                        nc.vector.tensor_tensor(yT[:], zB[2][:], c7[:], OP.mult)

                        # u2 = Pp - Mm = 2*u1 (SBUF only -> GpSimd)
                        u2 = ppool.tile([P, nt], f32, tag="u2", name=f"u2_{i}")
                        nc.gpsimd.tensor_tensor(u2[:], Pp[:], Mm[:], OP.subtract)
                        prods[i] = (yl, yT, u2)

                # ---------- phase B: layer-2 matvecs, w, reductions ----------
                for dj in range(HB):
                    for j in range(NDF):
                        i = NDF * (blk * HB + dj) + j
                        sl = slice(i * nt, (i + 1) * nt)
                        yl, yT, u2 = prods[i]
                        rT = rpool.tile([P, nt], f32, tag="r", name=f"rT_{i}")
                        nc.tensor.matmul(rT[:], CT_sb[:], yT[:],
                                         start=True, stop=True)
                        wT = spool.tile([P, nt], fmm, tag="wT")
                        nc.vector.scalar_tensor_tensor(
                            wT[:], rT[:], d_sb[:], yT[:], OP.add, OP.mult
                        )
                        rY = rpool.tile([P, nt], f32, tag="r", name=f"rY_{i}")
                        nc.tensor.matmul(rY[:], CT_sb[:], yl[:],
                                         start=True, stop=True)
                        wY = spool.tile([P, nt], fmm, tag="wY")
                        nc.vector.scalar_tensor_tensor(
                            wY[:], rY[:], d_sb[:], yl[:], OP.add, OP.mult
                        )
                        rG = rpool.tile([P, nt], f32, tag="r", name=f"rG_{i}")
                        nc.tensor.matmul(rG[:], S_sb[:], yl[:],
                                         start=True, stop=True)
                        wG = spool.tile([P, nt], fmm, tag="wG")
                        nc.vector.scalar_tensor_tensor(
                            wG[:], rG[:], d_sb[:], u2[:], OP.add, OP.mult
                        )

                        # reductions, col-tiled into one PSUM bank
                        psV = vpool.tile([P, nt], f32, tag="v", name=f"v_{i}")
                        nc.tensor.matmul(psV[0:32, :], onespad, wT[:],
                                         start=True, stop=True,
                                         tile_position=(0, 0))
                        nc.tensor.matmul(psV[32:64, :], onespad, wY[:],
                                         start=True, stop=True,
                                         tile_position=(0, 32))
                        nc.tensor.matmul(psV[64:96, :], halfpad, wG[:],
                                         start=True, stop=True,
                                         tile_position=(0, 64))
                        nc.scalar.activation(
                            vout[0:65, sl], psV[0:65, :],
                            AF.Identity, bias=ebias_sb[0:65, :],
                        )

                # stream this block's outputs while later blocks compute
                bsl = slice(blk * HB * NDF * nt, (blk + 1) * HB * NDF * nt)
                nc.sync.dma_start(vt_o[:, bsl], vout[0:1, bsl])
                nc.sync.dma_start(vy_o[:, bsl], vout[32:33, bsl])
                nc.sync.dma_start(vg_o[:, bsl], vout[64:65, bsl])

            # ---- center output DMA (main outputs stream per block) ----
            nc.sync.dma_start(vc_o, vcout[:])

    nc.compile()
    return nc


def _prep_inputs(T, l, l1_dot, center, w1_1, b1_1, w2_1, b2_1,
                 w1_2, b1_2, w2_2, b2_2, w_out,
                 ntiles=NTILES, nt=NT, ncenter=CPER):
    """Host-side shard + layout prep. Returns list of per-core input dicts."""
    import ml_dtypes
    f32 = np.float32
    bf = ml_dtypes.bfloat16
    NDF = 2
    nd, nt2 = ntiles // NDF, NDF * nt

    def bsplit(x):  # -> (hi, lo) bf16 arrays
        hi = x.astype(bf)
        lo = (x - hi.astype(f32)).astype(bf)
        return hi, lo

    def aug_t(x):  # (n, 8) -> (9, n) with ones row
        n = x.shape[0]
        out = np.empty((9, n), f32)
        out[0:8] = x.T
        out[8] = 1.0
        return out

    W1a = np.concatenate([w1_1, b1_1[:, None]], axis=1).astype(f32)  # (128, 9)
    W2a = np.concatenate([w2_1, b2_1[:, None]], axis=1).astype(f32)

    def wstack(W):  # (128, 9) -> (27, 128) bf16 [Wh; Wh; Wl]
        hi, lo = bsplit(W.T)
        return np.concatenate([hi, hi, lo], axis=0)

    w1s, w2s = wstack(W1a), wstack(W2a)
    wstk = np.stack([w1s, w2s, w1s, w2s])  # (4, 27, 128) bf16
    wvec = w_out.reshape(P, 1).astype(f32)
    b3v = b1_2.reshape(P, 1).astype(f32)
    b4v = b2_2.reshape(P, 1).astype(f32)
    ident = np.eye(P, dtype=f32)
    onesc = np.stack([np.ones(P, f32), np.full(P, 0.5, f32)], axis=1)
    maskr = np.zeros((1, P), f32)
    maskr[0, :64] = 1.0
    redw = np.zeros((P, 64), f32)
    redw[:, 0] = 1.0
    redw[:, 32] = 0.5

    per_core = ntiles * nt
    in_maps = []
    for c in range(NCORES):
        sl = slice(c * per_core, (c + 1) * per_core)
        xt = aug_t(T[sl])
        xl = aug_t(l[sl])
        xp = aug_t(l[sl] + l1_dot[sl])
        xm = aug_t(l[sl] - l1_dot[sl])

        def tiles(x):  # (9, per_core) -> (nd, 27, nt2) bf16 [xh; xl; xh]
            hi, lo = bsplit(x)
            st = np.concatenate([hi, lo, hi], axis=0)  # (27, per_core)
            return np.ascontiguousarray(
                st.reshape(27, nd, nt2).transpose(1, 0, 2)
            )

        xt_t, xl_t, xp_t, xm_t = tiles(xt), tiles(xl), tiles(xp), tiles(xm)
        stkA = np.ascontiguousarray(
            np.stack([xl_t, xl_t, xp_t, xp_t], axis=1)
        )  # (nd, 4, 9, nt2)
        stkB = np.ascontiguousarray(np.stack([xm_t, xm_t, xt_t, xt_t], axis=1))
        ctr_f = aug_t(center[c * ncenter:(c + 1) * ncenter])
        ch, cl = bsplit(ctr_f)
        ctr = np.ascontiguousarray(np.concatenate([ch, cl, ch], axis=0))

        in_maps.append({
            "stkA": stkA, "stkB": stkB, "wstk": wstk,
            "amat": w1_2.astype(f32), "bmat": w2_2.astype(f32),
            "wvec": wvec, "b3v": b3v, "b4v": b4v, "ident": ident,
            "onesc": onesc, "maskr": maskr, "redw": redw, "ctr": ctr,
        })
    return in_maps


_NC_CACHE = {}


def _get_nc(key=(NTILES, NT, CPER)):
    if key not in _NC_CACHE:
        _NC_CACHE[key] = _build_program(*key)
    return _NC_CACHE[key]


def _run(inputs, trace=False, tmpdir=None):
    from concourse.bass_utils import run_bass_kernel_spmd

    nc = _get_nc()
    in_maps = _prep_inputs(**inputs)
    res = run_bass_kernel_spmd(
        nc, in_maps, list(range(NCORES)), trace=trace, tmpdir=tmpdir
    )
    vt = np.concatenate([r["vt_o"].reshape(-1) for r in res.results])
    vy = np.concatenate([r["vy_o"].reshape(-1) for r in res.results])
    vg = np.concatenate([r["vg_o"].reshape(-1) for r in res.results])
    vc = np.concatenate([r["vc_o"].reshape(-1) for r in res.results])
    out = (
        vt.reshape(BATCH, 1).astype(np.float32),
        vy.reshape(BATCH, 1).astype(np.float32),
        vg.astype(np.float32),
        vc.reshape(512, 1).astype(np.float32),
    )
    return out, res


def kernel(**inputs):
    inputs = {k: np.asarray(v, dtype=np.float32) for k, v in inputs.items()}
    out, _ = _run(inputs, trace=False)
    return out
